# revision 22
# baseline (speedup 1.0000x reference)
"""Trainium2 Bass kernel for nn_GammaSpaceLayer.

The reference is an SSM: fixed "gamma" transition A (bidiagonal), bilinear
discretization, kernel k[m] = C dA^m dB, then FFT causal conv + D*u skip.
This is computed here as a chunked linear scan (state dim H=64, chunk T=8):

  per (batch, chunk c, local t):  x = sum_{s<=t} dA^{t-s} dB u[c,s]  (intra)
                                  + dA^{t+1} xend[c-1]               (inter)
  y = C x + D*u

All heavy work is matmuls on the PE array in a transposed layout
(contraction dims on SBUF partitions).  Data-parallel over batch: 16
batches over 8 cores = 2 per core.  Small input-dependent matrices
(powers of dA) are precomputed on host in float64 and passed as inputs,
so the Bass program is input-independent (NEFF cache friendly).
"""

import numpy as np

import concourse.bass as bass
import concourse.mybir as mybir
import concourse.tile as tile
from concourse.vector_clock import ScopedClock
from concourse.bass_utils import run_bass_kernel_spmd

# problem constants (hardcoded per contract)
H, S = 64, 128          # state dim, io channel dim
B, L = 16, 2048         # full batch, seq len
N_CORES = 8
PB = B // N_CORES       # batches per core (2)
T = 8                   # chunk length
C = L // T              # chunks per batch (256)
SCAN_RADIX = 4          # radix-4 Hillis-Steele scan over chunks
SCAN_LEVELS = 4         # 4^4 = 256 = C
NWP = SCAN_LEVELS * (SCAN_RADIX - 1)   # 12 scan weight matrices
COLS = PB * C           # (b, c) columns per core (512)
DT_MIN, DT_MAX = 0.001, 0.1

F32 = mybir.dt.float32
MM = mybir.dt.float32r  # matmul operand dtype: fp32 bits, fast PE mode (1cy/row at N>=256)


class _TC(tile.TileContext):
    """TileContext whose tail drain splits multi-sem waits: this walrus
    build caps CTRL instructions at one sync-wait command."""

    def _drain_and_barrier(self, tick_clock, wait_clock):
        probe = self.nc.sync.drain()
        wait_clock.add_sem_waits(probe.ins, ScopedClock({None: tick_clock.global_clock}))
        si = probe.ins.sync_info
        if si is not None and si.on_wait and len(si.on_wait) > 1:
            waits = list(si.on_wait)
            probe.ins.sync_info = mybir.SyncInfo(
                on_wait=[waits[0]], on_update=list(si.on_update or []))
            for w in waits[1:]:
                d = self.nc.sync.drain()
                d.ins.sync_info = mybir.SyncInfo(on_wait=[w], on_update=[])
        self.nc.all_engine_barrier()
        assert self.sems is not None
        popped = self.nc._tile_sem_poison_stack.pop()
        assert popped is self._sem_poison
        self.nc.clear_and_free_semaphores(list(self.sems.allocated().values()))
        self.nc.all_engine_barrier()


def _split_multi_waits(nc):
    """This walrus build allows only ONE sync-wait command per instruction.
    Split extras onto same-engine InstEventSemaphore carriers inserted
    immediately before (engine program order preserves semantics)."""
    n = 0
    for f in nc.m.functions:
        for b in f.blocks:
            il = b.instructions
            i = 0
            while i < len(il):
                ins = il[i]
                si = ins.sync_info
                if si is not None and si.on_wait and len(si.on_wait) > 1:
                    waits = list(si.on_wait)
                    ins.sync_info = mybir.SyncInfo(
                        on_wait=[waits[-1]], on_update=list(si.on_update or []))
                    for j, w in enumerate(waits[:-1]):
                        ev = mybir.InstEventSemaphore(
                            name=f"{ins.name}_wsplit{j}", ins=[], outs=[])
                        ev.engine = ins.engine
                        ev.sync_info = mybir.SyncInfo(on_wait=[w], on_update=[])
                        il.insert(i, ev)
                        i += 1
                        n += 1
                i += 1
    return n


def _build():
    nc = bass.Bass()
    u_d = nc.dram_tensor("u", [PB, L, S], MM, kind="ExternalInput")
    gt_d = nc.dram_tensor("GT", [T, S, H], MM, kind="ExternalInput")      # (dA^m dB)^T
    ap_d = nc.dram_tensor("APOWT", [T, H, H], MM, kind="ExternalInput")   # (dA^{t+1})^T
    wp_d = nc.dram_tensor("WPT", [NWP, H, H], MM, kind="ExternalInput")   # scan weights^T
    ct_d = nc.dram_tensor("CT2", [H, S], MM, kind="ExternalInput")        # C^T
    dd_d = nc.dram_tensor("DD", [S, S], MM, kind="ExternalInput")         # diag(D)
    eye_d = nc.dram_tensor("EYE", [128, 128], MM, kind="ExternalInput")
    zz_d = nc.dram_tensor("ZZ", [H, COLS], MM, kind="ExternalInput")      # zeros
    y_d = nc.dram_tensor("y", [PB, L, S], F32, kind="ExternalOutput")

    NTILE = PB * L // 128  # 32 row-tiles of u/y

    with _TC(nc) as tc:
        with (
            tc.tile_pool(name="const", bufs=1) as cpool,
            tc.tile_pool(name="big", bufs=1) as bigpool,
            tc.tile_pool(name="stage", bufs=6) as stpool,
            tc.tile_pool(name="ostage", bufs=6) as ostpool,
            tc.tile_pool(name="psum_t", bufs=3, space="PSUM") as pst,
            tc.tile_pool(name="psum_z", bufs=3, space="PSUM") as psz,
            tc.tile_pool(name="psum_s", bufs=1, space="PSUM") as pss,
            tc.tile_pool(name="psum_y", bufs=1, space="PSUM") as psy,
        ):
            ident = cpool.tile([128, 128], MM)
            nc.sync.dma_start(ident[:], eye_d[:])

            gt_sb = cpool.tile([S, T * H], MM)
            nc.sync.dma_start(gt_sb[:].rearrange("p (m n) -> p m n", m=T),
                              gt_d[:].rearrange("m p n -> p m n"))
            ap_sb = cpool.tile([H, T * H], MM)
            nc.sync.dma_start(ap_sb[:].rearrange("p (m n) -> p m n", m=T),
                              ap_d[:].rearrange("m p n -> p m n"))
            wp_sb = cpool.tile([H, NWP * H], MM)
            nc.sync.dma_start(wp_sb[:].rearrange("p (m n) -> p m n", m=NWP),
                              wp_d[:].rearrange("m p n -> p m n"))
            ct_sb = cpool.tile([H, S], MM)
            nc.sync.dma_start(ct_sb[:], ct_d[:])
            dd_sb = cpool.tile([S, S], MM)
            nc.sync.dma_start(dd_sb[:], dd_d[:])

            # ---- load u and transpose on chip:  uT[i, b*L + l] ----
            uT = bigpool.tile([S, PB * L], MM)
            u_flat = u_d[:].rearrange("b l i -> (b l) i")
            for it in range(NTILE):
                st = stpool.tile([128, S], MM, tag="ustage")
                nc.sync.dma_start(st[:], u_flat[it * 128:(it + 1) * 128, :])
                pt = pst.tile([S, 128], MM, tag="tp")
                nc.tensor.transpose(pt[:], st[:], ident[:])
                eng = nc.scalar.copy if it % 2 else nc.vector.tensor_copy
                eng(uT[:, it * 128:(it + 1) * 128], pt[:])

            uT_r = uT[:].rearrange("p (b c t) -> p b c t", b=PB, c=C, t=T)

            # ---- stage 1 (t = T-1 only): b_c = ZT[T-1] ----
            zlast = psz.tile([H, COLS], F32, tag="z")
            for s in range(T):
                nc.tensor.matmul(
                    zlast[:], gt_sb[:, (T - 1 - s) * H:(T - s) * H], uT_r[:, :, :, s],
                    start=(s == 0), stop=(s == T - 1))

            # ---- chunk-state log-scan (Hillis-Steele with matrix weights) ----
            # layout per batch: [C zero cols | C data cols]
            sc_a = bigpool.tile([H, 2 * PB * C], MM)
            sc_b = bigpool.tile([H, 2 * PB * C], MM)
            sa_r = sc_a[:].rearrange("p (b x) -> p b x", b=PB)
            sb_r = sc_b[:].rearrange("p (b x) -> p b x", b=PB)
            nc.sync.dma_start(sa_r[:, :, 0:C], zz_d[:].rearrange("p (b c) -> p b c", b=PB))
            nc.sync.dma_start(sb_r[:, :, 0:C], zz_d[:].rearrange("p (b c) -> p b c", b=PB))
            nc.vector.tensor_copy(
                sa_r[:, :, C:2 * C], zlast[:].rearrange("p (b c) -> p b c", b=PB))
            cur, nxt = (sc_a, sa_r), (sc_b, sb_r)
            for d in range(SCAN_LEVELS):
                step = SCAN_RADIX ** d
                ps = pss.tile([H, COLS], F32, tag="scan")
                for k in range(1, SCAN_RADIX):
                    sh = k * step
                    w = d * (SCAN_RADIX - 1) + (k - 1)
                    nc.tensor.matmul(
                        ps[:], wp_sb[:, w * H:(w + 1) * H],
                        cur[1][:, :, C - sh:2 * C - sh],
                        start=(k == 1), stop=(k == SCAN_RADIX - 1))
                nc.vector.tensor_add(nxt[1][:, :, C:2 * C],
                                     ps[:].rearrange("p (b c) -> p b c", b=PB),
                                     cur[1][:, :, C:2 * C])
                cur, nxt = nxt, cur
            xend_r = cur[1]          # (H, b, 2C): data at [C:2C], zeros before
            # xprev_c = xend_{c-1}: shift right by one chunk
            xprev = xend_r[:, :, C - 1:2 * C - 1]   # (H, PB, C)

            # ---- per-t: Z/X then y ----
            xt_sb = bigpool.tile([H, T * COLS], MM)
            yT = bigpool.tile([S, PB * L], MM)
            yT_r = yT[:].rearrange("p (b c t) -> p b c t", b=PB, c=C, t=T)
            for t in range(T):
                z = psz.tile([H, COLS], F32, tag="z")
                for s in range(t + 1):
                    nc.tensor.matmul(
                        z[:], gt_sb[:, (t - s) * H:(t - s + 1) * H], uT_r[:, :, :, s],
                        start=(s == 0), stop=False)
                nc.tensor.matmul(z[:], ap_sb[:, t * H:(t + 1) * H], xprev,
                                 start=False, stop=True)
                eng = nc.scalar.copy if t % 2 else nc.vector.tensor_copy
                eng(xt_sb[:, t * COLS:(t + 1) * COLS], z[:])

                yp = psy.tile([S, COLS], F32, tag="y")
                nc.tensor.matmul(yp[:], ct_sb[:], xt_sb[:, t * COLS:(t + 1) * COLS],
                                 start=True, stop=False)
                nc.tensor.matmul(yp[:], dd_sb[:], uT_r[:, :, :, t],
                                 start=False, stop=True)
                eng = nc.scalar.copy if t % 2 else nc.vector.tensor_copy
                eng(yT_r[:, :, :, t], yp[:].rearrange("p (b c) -> p b c", b=PB))

            # ---- transpose back & store:  y[b, l, o] ----
            # yT col = b*L + l  (already l-ordered)
            y_flat = y_d[:].rearrange("b l i -> (b l) i")
            for it in range(NTILE):
                pt = pst.tile([128, S], MM, tag="tp")
                nc.tensor.transpose(
                    pt[:], yT[:, it * 128:(it + 1) * 128], ident[:])
                ot = ostpool.tile([128, S], F32, tag="ostage")
                eng = nc.scalar.copy if it % 2 else nc.vector.tensor_copy
                eng(ot[:], pt[:])
                nc.sync.dma_start(y_flat[it * 128:(it + 1) * 128, :], ot[:])

    _split_multi_waits(nc)
    return nc


_NC_CACHE = {}


def _get_nc():
    if "nc" not in _NC_CACHE:
        _NC_CACHE["nc"] = _build()
    return _NC_CACHE["nc"]


def _host_precompute(Bmat, Cmat, Dvec, log_dt):
    Bm = np.asarray(Bmat, dtype=np.float64)
    x = np.float64(log_dt)
    dt = np.clip(np.logaddexp(0.0, x), DT_MIN, DT_MAX)   # softplus, clipped
    A = -np.eye(H) + np.eye(H, k=-1)
    back = np.eye(H) - 0.5 * dt * A
    fwd = np.eye(H) + 0.5 * dt * A
    dA = np.linalg.solve(back, fwd)
    dB = np.linalg.solve(back, dt * Bm)                  # (H, S)
    G = [dB]
    for _ in range(1, T):
        G.append(dA @ G[-1])
    dApow = [dA]
    for _ in range(1, T):
        dApow.append(dA @ dApow[-1])
    A8 = dApow[T - 1]
    Wp = []
    for d in range(SCAN_LEVELS):
        for k in range(1, SCAN_RADIX):
            Wp.append(np.linalg.matrix_power(A8, k * SCAN_RADIX ** d))
    f32 = lambda a: np.ascontiguousarray(a, dtype=np.float32)
    return {
        "GT": f32(np.stack([g.T for g in G])),                 # (T, S, H)
        "APOWT": f32(np.stack([p.T for p in dApow])),          # (T, H, H)
        "WPT": f32(np.stack([w.T for w in Wp])),               # (NWP, H, H)
        "CT2": f32(np.asarray(Cmat, dtype=np.float64).T),      # (H, S)
        "DD": f32(np.diag(np.asarray(Dvec, dtype=np.float64))),
        "EYE": f32(np.eye(128)),
        "ZZ": f32(np.zeros((H, COLS))),
    }


def kernel(u, B, C, D, log_dt, _trace=False):
    u = np.ascontiguousarray(u, dtype=np.float32)
    pre = _host_precompute(B, C, D, log_dt)
    nc = _get_nc()
    in_maps = [{"u": u[k * PB:(k + 1) * PB], **pre} for k in range(N_CORES)]
    res = run_bass_kernel_spmd(nc, in_maps, core_ids=list(range(N_CORES)),
                               trace=_trace)
    y = np.concatenate([res.results[k]["y"] for k in range(N_CORES)], axis=0)
    if _trace:
        kernel.last_result = res
    return y
